# revision 41
# baseline (speedup 1.0000x reference)
"""Multi-head attention Bass/Tile kernel for TRN2, 8-core SPMD.

Sharding: core c handles batch b = c//2 and head-group g = c%2 (6 of 12 heads).
Each core computes its 6 heads end-to-end plus a partial output projection
(over its 384 of 768 ctx dims); the host sums the two partials per batch.

v6 design notes (all calibrated against perfetto traces):
- ScalarE exp is a ~200us serial floor; PE (all matmuls) is ~260us busy.
  The kernel is one fused PE stream ordered so exp starts ~15us in and both
  engines stay saturated: minimal prefix (k-chunk m=0/block0 + q m=0/s0),
  then a flat (s-block, head-pair) attention loop with all other projections
  woven in as "extras" at units where their outputs are first needed.
- 2-group exp pipeline lag: ctx(g-2) is emitted after sc(g), so the PE never
  waits on the exp latency (PE p-state: any stall drops the tensor clock
  2-3.7x; keeping it saturated is worth more than any local reordering).
- DMA queues are descriptor-bound (~78ns per row-descriptor): Wq|Wk|Wv are
  packed host-side into one [E, 1152] tensor (4.5KB rows), hs col-blocks 1-3
  load as [128, 1536] tiles (6KB rows) — inputs land by ~15us.
- PSUM accumulation chains serialize ~50-90ns per link; independent chains
  are emitted pairwise interleaved to hide it.
- Normalize: reciprocal_approx_fast (single DVE op, ~18 bits) + gpsimd
  partition_broadcast; exp/v_aug in bf16 (same PE rate, half the SBUF).
"""

from contextlib import ExitStack

import ml_dtypes
import numpy as np

import concourse.bass as bass
import concourse.tile as tile
from concourse import bacc, mybir
from concourse._compat import with_exitstack

F32R = mybir.dt.float32r
F32 = mybir.dt.float32
BF16 = mybir.dt.bfloat16
AF = mybir.ActivationFunctionType

B, E, S, H, D = 4, 768, 2048, 12, 64
NH = 6          # heads per core
HD = NH * D     # 384 head-dims per core
NE = E // 128   # 6 e-chunks
NM = HD // 128  # 3 m-chunks (2 heads each)
NT = S // 128   # 16 t-tiles
SBW = 512       # s-block width
NS = S // SBW   # 4 s-blocks
VW = 96         # v_aug width: col 0 = ones (denominator), 32..95 = v-dims
WQO, WKO, WVO = 0, HD, 2 * HD  # column offsets in packed wqkv


@with_exitstack
def mha_tile(ctx: ExitStack, tc, hs, wqkv, biases, woT, outT):
    nc = tc.nc

    persist = ctx.enter_context(tc.tile_pool(name="persist", bufs=1))

    # --- persistent tiles ---
    wqkv_sb = [persist.tile([128, 3 * HD], BF16, name=f"wqkv{e}") for e in range(NE)]
    woT_sb = [persist.tile([128, E], BF16, name=f"wo{f}") for f in range(NM)]
    hs0_sb = [persist.tile([128, SBW], BF16, name=f"hs0_{e}") for e in range(NE)]
    hs123_sb = [persist.tile([128, 3 * SBW], BF16, name=f"hs123_{e}") for e in range(NE)]
    # host-packed [128, 397]: bq(3) | bk(3) | bo/2(6) | bv-bcast(384) | zero(1)
    bias_sb = persist.tile([128, 397], F32, name="bias")
    bq_sb = bias_sb[:, 0:NM]
    bk_sb = bias_sb[:, NM : 2 * NM]
    bo_sb = bias_sb[:, 2 * NM : 2 * NM + NE]
    bv_bc = bias_sb[:, 12 : 12 + HD]
    zb = bias_sb[:, 396:397]
    po_sb = [persist.tile([128, SBW], BF16, name=f"po{et}") for et in range(NE)]

    kT_sb = [persist.tile([128, S], F32R, name=f"kT{m}") for m in range(NM)]
    qT_sb = [
        [persist.tile([128, SBW], F32R, name=f"qT{m}_{s}") for s in range(NS)]
        for m in range(NM)
    ]
    ctxT_sb = [
        [persist.tile([128, SBW], BF16, name=f"ctxT{m}_{s}") for s in range(NS)]
        for m in range(NM)
    ]
    v_aug = [persist.tile([128, NH, VW], BF16, name=f"vaug{t}") for t in range(NT)]

    def hs_at(e, s):  # moving [128, 512] for k/q of s-block s
        return hs0_sb[e][:] if s == 0 else hs123_sb[e][:, SBW * (s - 1) : SBW * s]

    def hs_tile_at(e, t):  # stationary [128, 128] for v of t-tile t
        if t < 4:
            return hs0_sb[e][:, 128 * (t % 4) : 128 * (t % 4 + 1)]
        o = 128 * (t - 4)
        return hs123_sb[e][:, o : o + 128]

    # --- DMA issue order = need order (biases first: they gate the PSUM
    # evacuations of the very first projection chunks) ---
    nc.sync.dma_start(bias_sb[:], biases[:, :])
    for e in range(NE):
        esl = slice(128 * e, 128 * (e + 1))
        nc.sync.dma_start(wqkv_sb[e][:], wqkv[esl, :])
        nc.sync.dma_start(hs0_sb[e][:], hs[esl, 0:SBW])
    for e in range(NE):
        nc.sync.dma_start(
            hs123_sb[e][:], hs[128 * e : 128 * (e + 1), SBW:S]
        )
    for f in range(NM):
        nc.sync.dma_start(woT_sb[f][:], woT[128 * f : 128 * (f + 1), :])

    # v_aug init on DVE (runs during initial DMA wait)
    for t in range(NT):
        nc.vector.memset(v_aug[t][:, :, 0:32], 0.0)
        nc.vector.memset(v_aug[t][:, :, 0:1], 1.0)

    # --- pools ---
    pssc = ctx.enter_context(tc.tile_pool(name="pssc", bufs=2, space="PSUM"))
    psctx = ctx.enter_context(tc.tile_pool(name="psctx", bufs=1, space="PSUM"))
    ps3 = ctx.enter_context(tc.tile_pool(name="ps3", bufs=2, space="PSUM"))
    expp = ctx.enter_context(tc.tile_pool(name="expp", bufs=4))
    smp = ctx.enter_context(tc.tile_pool(name="smp", bufs=1))
    outp = ctx.enter_context(tc.tile_pool(name="outp", bufs=2))

    # --- projection chunks as (steps, finish) for pairwise interleaving of
    # independent PSUM accumulation chains (hides the per-link serialization)
    def kq_chunk(kind, m, s):
        off = WQO if kind == "q" else WKO
        msl = slice(off + 128 * m, off + 128 * (m + 1))
        pp = ps3.tile([128, SBW], F32, tag="o")
        steps = [
            (lambda e=e: nc.tensor.matmul(
                pp[:], wqkv_sb[e][:, msl], hs_at(e, s),
                start=(e == 0), stop=(e == NE - 1),
            ))
            for e in range(NE)
        ]

        def finish():
            if kind == "q":
                nc.vector.tensor_scalar_add(
                    out=qT_sb[m][s][:], in0=pp[:], scalar1=bq_sb[:, m : m + 1]
                )
            else:
                nc.vector.tensor_scalar_add(
                    out=kT_sb[m][:, SBW * s : SBW * (s + 1)], in0=pp[:],
                    scalar1=bk_sb[:, m : m + 1],
                )

        return steps, finish

    def v_chunk(t):
        pp = ps3.tile([128, SBW], F32, tag="o")
        steps = [
            (lambda e=e: nc.tensor.matmul(
                pp[:, 0:HD], hs_tile_at(e, t),
                wqkv_sb[e][:, WVO : WVO + HD],
                start=(e == 0), stop=(e == NE - 1),
            ))
            for e in range(NE)
        ]

        def finish():
            nc.vector.tensor_add(
                out=v_aug[t][:, :, 32 : 32 + D],
                in0=pp[:, 0:HD].rearrange("p (h d) -> p h d", h=NH),
                in1=bv_bc[:].rearrange("p (h d) -> p h d", h=NH),
            )

        return steps, finish

    def out_chunk(s, et):
        esl = slice(128 * et, 128 * (et + 1))
        pp = ps3.tile([128, SBW], F32, tag="o")
        steps = [
            (lambda f=f: nc.tensor.matmul(
                pp[:], woT_sb[f][:, esl], ctxT_sb[f][s][:],
                start=(f == 0), stop=(f == NM - 1),
            ))
            for f in range(NM)
        ]

        def finish():
            # evacuate on ScalarE (Identity+bias): keeps the DVE clear of the
            # normalize bursts so the ps3 rotation never stalls the PE
            ob = outp.tile([128, SBW], F32, tag="ob")
            nc.scalar.activation(
                ob[:], pp[:], AF.Identity, bias=bo_sb[:, et : et + 1]
            )
            nc.sync.dma_start(outT[esl, SBW * s : SBW * (s + 1)], ob[:])

        return steps, finish

    def out01_chunk(et):
        # partial out-proj of the LAST s-block over pairs 0-1 (normalized
        # long before pair 2): shrinks the post-last-exp tail to one matmul
        # plus one fused DVE op per e-chunk
        esl = slice(128 * et, 128 * (et + 1))
        pp = ps3.tile([128, SBW], F32, tag="o")
        steps = [
            (lambda f=f: nc.tensor.matmul(
                pp[:], woT_sb[f][:, esl], ctxT_sb[f][NS - 1][:],
                start=(f == 0), stop=(f == 1),
            ))
            for f in range(2)
        ]

        def finish():
            nc.vector.tensor_copy(po_sb[et][:], pp[:])

        return steps, finish

    def out2_chunk(et, alt_pool=False):
        esl = slice(128 * et, 128 * (et + 1))
        if alt_pool:  # pssc is idle at the tail; use it to deepen the pipe
            scp = pssc.tile([128, 2 * SBW], F32, tag="sc", name="scp")
            pp = scp[:, 0:SBW]
        else:
            pp = ps3.tile([128, SBW], F32, tag="o")
        steps = [
            lambda: nc.tensor.matmul(
                pp[:], woT_sb[2][:, esl], ctxT_sb[2][NS - 1][:],
                start=True, stop=True,
            )
        ]

        def finish():
            ob = outp.tile([128, SBW], F32, tag="ob")
            nc.vector.scalar_tensor_tensor(
                out=ob[:], in0=pp[:], scalar=bo_sb[:, et : et + 1],
                in1=po_sb[et][:],
                op0=mybir.AluOpType.add, op1=mybir.AluOpType.add,
            )
            nc.sync.dma_start(
                outT[esl, SBW * (NS - 1) : S], ob[:]
            )

        return steps, finish

    def emit_group(*chunks):
        built = [c() if callable(c) else c for c in chunks]
        n = max(len(st) for st, _ in built)
        for i in range(n):
            for st, _ in built:
                if i < len(st):
                    st[i]()
        for _, fin in built:
            fin()

    ITEMS = [(s, p) for s in range(NS) for p in range(NM)]
    ctx_ps = {}

    def emit_sc_exp(i, g):
        s, p = ITEMS[i]
        tsl = slice(128 * g, 128 * (g + 1))
        sc = pssc.tile([128, 2 * SBW], F32, tag="sc")
        nc.tensor.matmul(
            sc[:, 0:SBW], kT_sb[p][0:D, tsl], qT_sb[p][s][0:D, :],
            start=True, stop=True,
        )
        nc.tensor.matmul(
            sc[:, SBW : 2 * SBW], kT_sb[p][D:128, tsl], qT_sb[p][s][D:128, :],
            start=True, stop=True,
        )
        ex = expp.tile([128, 2 * SBW], BF16, tag="ex")
        nc.scalar.activation(ex[:], sc[:], AF.Exp, bias=zb)
        return ex

    def emit_ctx(i, g, ex):
        s, p = ITEMS[i]
        if g == 0:
            cpA = psctx.tile([128, SBW], F32, tag="cA", name="cpA")
            cpB = psctx.tile([128, SBW], F32, tag="cB", name="cpB")
            ctx_ps[i] = (cpA, cpB)
        cpA, cpB = ctx_ps[i]
        st, sp = (g == 0), (g == NT - 1)
        nc.tensor.matmul(
            cpA[0:VW, :], v_aug[g][:, 2 * p, :], ex[:, 0:SBW], start=st, stop=sp
        )
        nc.tensor.matmul(
            cpB[0:VW, :], v_aug[g][:, 2 * p + 1, :], ex[:, SBW : 2 * SBW],
            start=st, stop=sp,
        )

    def normalize(i, last=False):
        # A/B-interleaved emission: copies free the ctx PSUM early, and the
        # gpsimd broadcasts overlap the DVE reciprocal/multiply chain. For
        # the final pair there is no next consumer of the ctx PSUM, so skip
        # the copies and read PSUM directly (shortens the kernel tail).
        s, p = ITEMS[i]
        cpA, cpB = ctx_ps.pop(i)
        src = {}
        bcs = {}
        if last:
            src = {"A": cpA, "B": cpB}
        else:
            for tg, cp in (("A", cpA), ("B", cpB)):
                src[tg] = smp.tile([VW, SBW], F32, tag=f"cu{tg}", name="cu")
                nc.vector.tensor_copy(src[tg][:], cp[0:VW, :])
        for tg in ("A", "B"):
            rec = smp.tile([1, SBW], F32, tag=f"rec{tg}", name="rec")
            nc.vector.reciprocal_approx_fast(out=rec[0:1, :], in_=src[tg][0:1, :])
            bcs[tg] = smp.tile([VW, SBW], F32, tag=f"bcs{tg}", name="bcs")
            nc.gpsimd.partition_broadcast(bcs[tg][:], rec[0:1, :])
        mul = nc.vector.tensor_mul
        for hh, tg in ((2 * p, "A"), (2 * p + 1, "B")):
            o = D * (hh % 2)
            for q in range(2):  # 32-partition chunks (partition shift rules)
                mul(
                    out=ctxT_sb[p][s][o + 32 * q : o + 32 * (q + 1), :],
                    in0=src[tg][32 + 32 * q : 64 + 32 * q, :],
                    in1=bcs[tg][32 + 32 * q : 64 + 32 * q, :],
                )

    # --- weave schedule: extras[(item, g)] emitted after sc(g), before the
    # lag-2 ctx drain. Every chunk is placed just ahead of its first consumer
    # so exp starts ~15us in and the PE stays saturated throughout. ---
    extras = {}

    def put(i, g, *specs):
        extras.setdefault((i, g), []).extend(specs)

    # item 0: v tiles (ctx consumes v[t] at unit t+2), k(0, b) ahead of
    # sc unit 4b, then k(1,0)/q(1,0) for item 1
    put(0, 0, lambda: v_chunk(0), lambda: v_chunk(1))
    put(0, 2, lambda: v_chunk(2), lambda: kq_chunk("k", 0, 1))
    put(0, 3, lambda: v_chunk(3))
    put(0, 4, lambda: v_chunk(4), lambda: v_chunk(5))
    put(0, 6, lambda: v_chunk(6), lambda: kq_chunk("k", 0, 2))
    put(0, 7, lambda: v_chunk(7))
    put(0, 8, lambda: v_chunk(8), lambda: v_chunk(9))
    put(0, 10, lambda: v_chunk(10), lambda: kq_chunk("k", 0, 3))
    put(0, 11, lambda: v_chunk(11))
    put(0, 12, lambda: v_chunk(12), lambda: v_chunk(13))
    put(0, 14, lambda: v_chunk(14), lambda: v_chunk(15))
    put(0, 15, lambda: kq_chunk("k", 1, 0), lambda: kq_chunk("q", 1, 0))
    # item 1: k(1, 1..3), then k(2,0)/q(2,0) for item 2
    put(1, 2, lambda: kq_chunk("k", 1, 1))
    put(1, 6, lambda: kq_chunk("k", 1, 2))
    put(1, 10, lambda: kq_chunk("k", 1, 3))
    put(1, 15, lambda: kq_chunk("k", 2, 0), lambda: kq_chunk("q", 2, 0))
    # item 2: k(2, 1..3)
    put(2, 2, lambda: kq_chunk("k", 2, 1))
    put(2, 6, lambda: kq_chunk("k", 2, 2))
    put(2, 10, lambda: kq_chunk("k", 2, 3))
    # q(*, s+1) during item 3s+2
    for s in range(NS - 1):
        for m in range(NM):
            put(3 * s + 2, 3 + 4 * m, lambda m=m, s=s: kq_chunk("q", m, s + 1))
    # partial out-proj (pairs 0-1) of the last s-block during its last item
    for et in range(0, NE, 2):
        put(
            11, 4 + 2 * et,
            lambda et=et: out01_chunk(et),
            lambda et=et: out01_chunk(et + 1),
        )
    # out-proj(s-1) pairs during item 3s: normalize(3s-1) lands at unit 1 and
    # its DVE burst runs ~units 1-5, so start at unit 6 to keep the ps3
    # evacuations (also DVE) from stalling the rotation
    for s in range(1, NS):
        for et in range(0, NE, 2):
            put(
                3 * s, 6 + 2 * et,
                lambda s=s, et=et: out_chunk(s - 1, et),
                lambda s=s, et=et: out_chunk(s - 1, et + 1),
            )

    # --- minimal prefix: k chunk (m=0, block 0) + q chunk (m=0, s0) ---
    emit_group(lambda: kq_chunk("k", 0, 0), lambda: kq_chunk("q", 0, 0))

    # --- fused attention stream with 2-group exp lag across boundaries ---
    from collections import deque

    pend = deque()

    def drain_one(last=False):
        pi, pg, pex = pend.popleft()
        emit_ctx(pi, pg, pex)
        if pg == NT - 1:
            normalize(pi, last=last)

    for i in range(len(ITEMS)):
        for g in range(NT):
            ex = emit_sc_exp(i, g)
            specs = extras.get((i, g))
            if specs:
                emit_group(*specs)
            if len(pend) == 3:
                drain_one()
            pend.append((i, g, ex))
    while pend:
        drain_one(last=(len(pend) == 1))
    for et in range(0, NE, 2):
        emit_group(
            lambda et=et: out2_chunk(et),
            lambda et=et: out2_chunk(et + 1, alt_pool=True),
        )


def build_nc():
    nc = bacc.Bacc("TRN2", target_bir_lowering=False, debug=False)
    hs = nc.dram_tensor("hs", [E, S], BF16, kind="ExternalInput")
    wqkv = nc.dram_tensor("wqkv", [E, 3 * HD], BF16, kind="ExternalInput")
    biases = nc.dram_tensor("biases", [128, 397], F32, kind="ExternalInput")
    woT = nc.dram_tensor("woT", [HD, E], BF16, kind="ExternalInput")
    outT = nc.dram_tensor("outT", [E, S], F32, kind="ExternalOutput")

    with tile.TileContext(nc) as tc:
        mha_tile(
            tc,
            hs[:, :], wqkv[:, :], biases[:, :], woT[:, :], outT[:, :],
        )
    nc.compile()
    return nc


def make_core_inputs(inputs: dict) -> list[dict]:
    """Full inputs -> per-core input maps (core c: batch c//2, head-group c%2)."""
    hsf = np.ascontiguousarray(np.asarray(inputs["hidden_state"], dtype=np.float32))
    Wq = np.asarray(inputs["Wq"], dtype=np.float32)
    Wk = np.asarray(inputs["Wk"], dtype=np.float32)
    Wv = np.asarray(inputs["Wv"], dtype=np.float32)
    Wo = np.asarray(inputs["Wo"], dtype=np.float32)
    bq = np.asarray(inputs["bq"], dtype=np.float32)
    bk = np.asarray(inputs["bk"], dtype=np.float32)
    bv = np.asarray(inputs["bv"], dtype=np.float32)
    bo = np.asarray(inputs["bo"], dtype=np.float32)

    maps = []
    for c in range(8):
        b, g = c // 2, c % 2
        hsl = slice(NH * g, NH * (g + 1))
        fsl = slice(HD * g, HD * (g + 1))
        wq_c = Wq[hsl].transpose(1, 0, 2).reshape(E, HD)
        wk_c = Wk[hsl].transpose(1, 0, 2).reshape(E, HD)
        wv_c = Wv[hsl].transpose(1, 0, 2).reshape(E, HD)
        # packed bias tile [128, 397]: bq(3) | bk(3) | bo/2(6) | bv-bcast | 0
        bias_t = np.zeros((128, 397), dtype=np.float32)
        bias_t[:, 0:3] = bq[hsl].reshape(HD).reshape(3, 128).T
        bias_t[:, 3:6] = bk[hsl].reshape(HD).reshape(3, 128).T
        bias_t[:, 6:12] = (bo / 2.0).reshape(6, 128).T
        bias_t[:, 12 : 12 + HD] = bv[hsl].reshape(HD)[None, :]
        bf16 = ml_dtypes.bfloat16
        maps.append(
            {
                "hs": np.ascontiguousarray(hsf[b].astype(bf16)),
                "wqkv": np.ascontiguousarray(
                    np.concatenate([wq_c, wk_c, wv_c], axis=1).astype(bf16)
                ),
                "biases": bias_t,
                "woT": np.ascontiguousarray(Wo[:, fsl].T.astype(bf16)),
            }
        )
    return maps


def combine_outputs(core_outs: list) -> np.ndarray:
    """Per-core outT partials -> full [B, E, S] output."""
    return np.stack(
        [core_outs[2 * b]["outT"] + core_outs[2 * b + 1]["outT"] for b in range(B)]
    ).astype(np.float32)


from concourse.bass_utils import run_bass_kernel_spmd

N_CORES = 8
_NC_CACHE = None


def _get_nc():
    global _NC_CACHE
    if _NC_CACHE is None:
        _NC_CACHE = build_nc()
    return _NC_CACHE


def kernel(**inputs) -> np.ndarray:
    """Full-input entry point: shard across 8 cores, run, unshard."""
    maps = make_core_inputs(inputs)
    nc = _get_nc()
    res = run_bass_kernel_spmd(nc, maps, core_ids=list(range(N_CORES)))
    outs = res.results
    return np.stack(
        [outs[2 * b]["outT"] + outs[2 * b + 1]["outT"] for b in range(B)]
    ).astype(np.float32)


# revision 42
# speedup vs baseline: 1.0221x; 1.0221x over previous
"""Multi-head attention Bass/Tile kernel for TRN2, 8-core SPMD.

Sharding: core c handles batch b = c//2 and head-group g = c%2 (6 of 12 heads).
Each core computes its 6 heads end-to-end plus a partial output projection
(over its 384 of 768 ctx dims); the host sums the two partials per batch.

v6 design notes (all calibrated against perfetto traces):
- ScalarE exp is a ~200us serial floor; PE (all matmuls) is ~260us busy.
  The kernel is one fused PE stream ordered so exp starts ~15us in and both
  engines stay saturated: minimal prefix (k-chunk m=0/block0 + q m=0/s0),
  then a flat (s-block, head-pair) attention loop with all other projections
  woven in as "extras" at units where their outputs are first needed.
- 2-group exp pipeline lag: ctx(g-2) is emitted after sc(g), so the PE never
  waits on the exp latency (PE p-state: any stall drops the tensor clock
  2-3.7x; keeping it saturated is worth more than any local reordering).
- DMA queues are descriptor-bound (~78ns per row-descriptor): Wq|Wk|Wv are
  packed host-side into one [E, 1152] tensor (4.5KB rows), hs col-blocks 1-3
  load as [128, 1536] tiles (6KB rows) — inputs land by ~15us.
- PSUM accumulation chains serialize ~50-90ns per link; independent chains
  are emitted pairwise interleaved to hide it.
- Normalize: reciprocal_approx_fast (single DVE op, ~18 bits) + gpsimd
  partition_broadcast; exp/v_aug in bf16 (same PE rate, half the SBUF).
"""

from contextlib import ExitStack

import ml_dtypes
import numpy as np

import concourse.bass as bass
import concourse.tile as tile
from concourse import bacc, mybir
from concourse._compat import with_exitstack

F32R = mybir.dt.float32r
F32 = mybir.dt.float32
BF16 = mybir.dt.bfloat16
AF = mybir.ActivationFunctionType

B, E, S, H, D = 4, 768, 2048, 12, 64
NH = 6          # heads per core
HD = NH * D     # 384 head-dims per core
NE = E // 128   # 6 e-chunks
NM = HD // 128  # 3 m-chunks (2 heads each)
NT = S // 128   # 16 t-tiles
SBW = 512       # s-block width
NS = S // SBW   # 4 s-blocks
VW = 96         # v_aug width: col 0 = ones (denominator), 32..95 = v-dims
WQO, WKO, WVO = 0, HD, 2 * HD  # column offsets in packed wqkv


@with_exitstack
def mha_tile(ctx: ExitStack, tc, hs, wqkv, biases, woT, outT):
    nc = tc.nc

    persist = ctx.enter_context(tc.tile_pool(name="persist", bufs=1))

    # --- persistent tiles ---
    wqkv_sb = [persist.tile([128, 3 * HD], BF16, name=f"wqkv{e}") for e in range(NE)]
    woT_sb = [persist.tile([128, E], BF16, name=f"wo{f}") for f in range(NM)]
    hs0_sb = [persist.tile([128, SBW], BF16, name=f"hs0_{e}") for e in range(NE)]
    hs123_sb = [persist.tile([128, 3 * SBW], BF16, name=f"hs123_{e}") for e in range(NE)]
    # host-packed [128, 397]: bq(3) | bk(3) | bo/2(6) | bv-bcast(384) | zero(1)
    bias_sb = persist.tile([128, 397], F32, name="bias")
    bq_sb = bias_sb[:, 0:NM]
    bk_sb = bias_sb[:, NM : 2 * NM]
    bo_sb = bias_sb[:, 2 * NM : 2 * NM + NE]
    bv_bc = bias_sb[:, 12 : 12 + HD]
    zb = bias_sb[:, 396:397]
    po_sb = [persist.tile([128, SBW], BF16, name=f"po{et}") for et in range(NE)]

    kT_sb = [persist.tile([128, S], F32R, name=f"kT{m}") for m in range(NM)]
    qT_sb = [
        [persist.tile([128, SBW], F32R, name=f"qT{m}_{s}") for s in range(NS)]
        for m in range(NM)
    ]
    ctxT_sb = [
        [persist.tile([128, SBW], BF16, name=f"ctxT{m}_{s}") for s in range(NS)]
        for m in range(NM)
    ]
    v_aug = [persist.tile([128, NH, VW], BF16, name=f"vaug{t}") for t in range(NT)]

    def hs_at(e, s):  # moving [128, 512] for k/q of s-block s
        return hs0_sb[e][:] if s == 0 else hs123_sb[e][:, SBW * (s - 1) : SBW * s]

    def hs_tile_at(e, t):  # stationary [128, 128] for v of t-tile t
        if t < 4:
            return hs0_sb[e][:, 128 * (t % 4) : 128 * (t % 4 + 1)]
        o = 128 * (t - 4)
        return hs123_sb[e][:, o : o + 128]

    # --- DMA issue order = need order (biases first: they gate the PSUM
    # evacuations of the very first projection chunks) ---
    nc.sync.dma_start(bias_sb[:], biases[:, :])
    for e in range(NE):
        esl = slice(128 * e, 128 * (e + 1))
        nc.sync.dma_start(wqkv_sb[e][:], wqkv[esl, :])
        nc.sync.dma_start(hs0_sb[e][:], hs[esl, 0:SBW])
    for e in range(NE):
        nc.sync.dma_start(
            hs123_sb[e][:], hs[128 * e : 128 * (e + 1), SBW:S]
        )
    for f in range(NM):
        nc.sync.dma_start(woT_sb[f][:], woT[128 * f : 128 * (f + 1), :])

    # v_aug init on DVE (runs during initial DMA wait)
    for t in range(NT):
        nc.vector.memset(v_aug[t][:, :, 0:32], 0.0)
        nc.vector.memset(v_aug[t][:, :, 0:1], 1.0)

    # --- pools ---
    pssc = ctx.enter_context(tc.tile_pool(name="pssc", bufs=2, space="PSUM"))
    psctx = ctx.enter_context(tc.tile_pool(name="psctx", bufs=1, space="PSUM"))
    ps3 = ctx.enter_context(tc.tile_pool(name="ps3", bufs=2, space="PSUM"))
    expp = ctx.enter_context(tc.tile_pool(name="expp", bufs=4))
    smp = ctx.enter_context(tc.tile_pool(name="smp", bufs=1))
    outp = ctx.enter_context(tc.tile_pool(name="outp", bufs=2))

    # --- projection chunks as (steps, finish) for pairwise interleaving of
    # independent PSUM accumulation chains (hides the per-link serialization)
    def kq_chunk(kind, m, s):
        off = WQO if kind == "q" else WKO
        msl = slice(off + 128 * m, off + 128 * (m + 1))
        pp = ps3.tile([128, SBW], F32, tag="o")
        steps = [
            (lambda e=e: nc.tensor.matmul(
                pp[:], wqkv_sb[e][:, msl], hs_at(e, s),
                start=(e == 0), stop=(e == NE - 1),
            ))
            for e in range(NE)
        ]

        def finish():
            if kind == "q":
                nc.vector.tensor_scalar_add(
                    out=qT_sb[m][s][:], in0=pp[:], scalar1=bq_sb[:, m : m + 1]
                )
            else:
                nc.vector.tensor_scalar_add(
                    out=kT_sb[m][:, SBW * s : SBW * (s + 1)], in0=pp[:],
                    scalar1=bk_sb[:, m : m + 1],
                )

        return steps, finish

    def v_chunk(t):
        pp = ps3.tile([128, SBW], F32, tag="o")
        steps = [
            (lambda e=e: nc.tensor.matmul(
                pp[:, 0:HD], hs_tile_at(e, t),
                wqkv_sb[e][:, WVO : WVO + HD],
                start=(e == 0), stop=(e == NE - 1),
            ))
            for e in range(NE)
        ]

        def finish():
            nc.vector.tensor_add(
                out=v_aug[t][:, :, 32 : 32 + D],
                in0=pp[:, 0:HD].rearrange("p (h d) -> p h d", h=NH),
                in1=bv_bc[:].rearrange("p (h d) -> p h d", h=NH),
            )

        return steps, finish

    def out_chunk(s, et):
        esl = slice(128 * et, 128 * (et + 1))
        pp = ps3.tile([128, SBW], F32, tag="o")
        steps = [
            (lambda f=f: nc.tensor.matmul(
                pp[:], woT_sb[f][:, esl], ctxT_sb[f][s][:],
                start=(f == 0), stop=(f == NM - 1),
            ))
            for f in range(NM)
        ]

        def finish():
            ob = outp.tile([128, SBW], F32, tag="ob")
            nc.vector.tensor_scalar_add(
                out=ob[:], in0=pp[:], scalar1=bo_sb[:, et : et + 1]
            )
            nc.sync.dma_start(outT[esl, SBW * s : SBW * (s + 1)], ob[:])

        return steps, finish

    def out01_chunk(et):
        # partial out-proj of the LAST s-block over pairs 0-1 (normalized
        # long before pair 2): shrinks the post-last-exp tail to one matmul
        # plus one fused DVE op per e-chunk
        esl = slice(128 * et, 128 * (et + 1))
        pp = ps3.tile([128, SBW], F32, tag="o")
        steps = [
            (lambda f=f: nc.tensor.matmul(
                pp[:], woT_sb[f][:, esl], ctxT_sb[f][NS - 1][:],
                start=(f == 0), stop=(f == 1),
            ))
            for f in range(2)
        ]

        def finish():
            nc.vector.tensor_copy(po_sb[et][:], pp[:])

        return steps, finish

    def out2_chunk(et, alt_pool=False):
        esl = slice(128 * et, 128 * (et + 1))
        if alt_pool:  # pssc is idle at the tail; use it to deepen the pipe
            scp = pssc.tile([128, 2 * SBW], F32, tag="sc", name="scp")
            pp = scp[:, 0:SBW]
        else:
            pp = ps3.tile([128, SBW], F32, tag="o")
        steps = [
            lambda: nc.tensor.matmul(
                pp[:], woT_sb[2][:, esl], ctxT_sb[2][NS - 1][:],
                start=True, stop=True,
            )
        ]

        def finish():
            ob = outp.tile([128, SBW], F32, tag="ob")
            nc.vector.scalar_tensor_tensor(
                out=ob[:], in0=pp[:], scalar=bo_sb[:, et : et + 1],
                in1=po_sb[et][:],
                op0=mybir.AluOpType.add, op1=mybir.AluOpType.add,
            )
            nc.sync.dma_start(
                outT[esl, SBW * (NS - 1) : S], ob[:]
            )

        return steps, finish

    def emit_group(*chunks):
        built = [c() if callable(c) else c for c in chunks]
        n = max(len(st) for st, _ in built)
        for i in range(n):
            for st, _ in built:
                if i < len(st):
                    st[i]()
        for _, fin in built:
            fin()

    ITEMS = [(s, p) for s in range(NS) for p in range(NM)]
    ctx_ps = {}

    def emit_sc_exp(i, g):
        s, p = ITEMS[i]
        tsl = slice(128 * g, 128 * (g + 1))
        sc = pssc.tile([128, 2 * SBW], F32, tag="sc")
        nc.tensor.matmul(
            sc[:, 0:SBW], kT_sb[p][0:D, tsl], qT_sb[p][s][0:D, :],
            start=True, stop=True,
        )
        nc.tensor.matmul(
            sc[:, SBW : 2 * SBW], kT_sb[p][D:128, tsl], qT_sb[p][s][D:128, :],
            start=True, stop=True,
        )
        ex = expp.tile([128, 2 * SBW], BF16, tag="ex")
        nc.scalar.activation(ex[:], sc[:], AF.Exp, bias=zb)
        return ex

    def emit_ctx(i, g, ex):
        s, p = ITEMS[i]
        if g == 0:
            cpA = psctx.tile([128, SBW], F32, tag="cA", name="cpA")
            cpB = psctx.tile([128, SBW], F32, tag="cB", name="cpB")
            ctx_ps[i] = (cpA, cpB)
        cpA, cpB = ctx_ps[i]
        st, sp = (g == 0), (g == NT - 1)
        nc.tensor.matmul(
            cpA[0:VW, :], v_aug[g][:, 2 * p, :], ex[:, 0:SBW], start=st, stop=sp
        )
        nc.tensor.matmul(
            cpB[0:VW, :], v_aug[g][:, 2 * p + 1, :], ex[:, SBW : 2 * SBW],
            start=st, stop=sp,
        )

    def normalize(i, last=False):
        # A/B-interleaved emission: copies free the ctx PSUM early, and the
        # gpsimd broadcasts overlap the DVE reciprocal/multiply chain. For
        # the final pair there is no next consumer of the ctx PSUM, so skip
        # the copies and read PSUM directly (shortens the kernel tail).
        s, p = ITEMS[i]
        cpA, cpB = ctx_ps.pop(i)
        src = {}
        bcs = {}
        if last:
            src = {"A": cpA, "B": cpB}
        else:
            for tg, cp in (("A", cpA), ("B", cpB)):
                src[tg] = smp.tile([VW, SBW], F32, tag=f"cu{tg}", name="cu")
                nc.vector.tensor_copy(src[tg][:], cp[0:VW, :])
        for tg in ("A", "B"):
            rec = smp.tile([1, SBW], F32, tag=f"rec{tg}", name="rec")
            nc.vector.reciprocal_approx_fast(out=rec[0:1, :], in_=src[tg][0:1, :])
            bcs[tg] = smp.tile([VW, SBW], F32, tag=f"bcs{tg}", name="bcs")
            nc.gpsimd.partition_broadcast(bcs[tg][:], rec[0:1, :])
        mul = nc.vector.tensor_mul
        for hh, tg in ((2 * p, "A"), (2 * p + 1, "B")):
            o = D * (hh % 2)
            for q in range(2):  # 32-partition chunks (partition shift rules)
                mul(
                    out=ctxT_sb[p][s][o + 32 * q : o + 32 * (q + 1), :],
                    in0=src[tg][32 + 32 * q : 64 + 32 * q, :],
                    in1=bcs[tg][32 + 32 * q : 64 + 32 * q, :],
                )

    # --- weave schedule: extras[(item, g)] emitted after sc(g), before the
    # lag-2 ctx drain. Every chunk is placed just ahead of its first consumer
    # so exp starts ~15us in and the PE stays saturated throughout. ---
    extras = {}

    def put(i, g, *specs):
        extras.setdefault((i, g), []).extend(specs)

    # item 0: v tiles (ctx consumes v[t] at unit t+2), k(0, b) ahead of
    # sc unit 4b, then k(1,0)/q(1,0) for item 1
    put(0, 0, lambda: v_chunk(0), lambda: v_chunk(1))
    put(0, 2, lambda: v_chunk(2), lambda: kq_chunk("k", 0, 1))
    put(0, 3, lambda: v_chunk(3))
    put(0, 4, lambda: v_chunk(4), lambda: v_chunk(5))
    put(0, 6, lambda: v_chunk(6), lambda: kq_chunk("k", 0, 2))
    put(0, 7, lambda: v_chunk(7))
    put(0, 8, lambda: v_chunk(8), lambda: v_chunk(9))
    put(0, 10, lambda: v_chunk(10), lambda: kq_chunk("k", 0, 3))
    put(0, 11, lambda: v_chunk(11))
    put(0, 12, lambda: v_chunk(12), lambda: v_chunk(13))
    put(0, 14, lambda: v_chunk(14), lambda: v_chunk(15))
    put(0, 15, lambda: kq_chunk("k", 1, 0), lambda: kq_chunk("q", 1, 0))
    # item 1: k(1, 1..3), then k(2,0)/q(2,0) for item 2
    put(1, 2, lambda: kq_chunk("k", 1, 1))
    put(1, 6, lambda: kq_chunk("k", 1, 2))
    put(1, 10, lambda: kq_chunk("k", 1, 3))
    put(1, 15, lambda: kq_chunk("k", 2, 0), lambda: kq_chunk("q", 2, 0))
    # item 2: k(2, 1..3)
    put(2, 2, lambda: kq_chunk("k", 2, 1))
    put(2, 6, lambda: kq_chunk("k", 2, 2))
    put(2, 10, lambda: kq_chunk("k", 2, 3))
    # q(*, s+1) during item 3s+2
    for s in range(NS - 1):
        for m in range(NM):
            put(3 * s + 2, 3 + 4 * m, lambda m=m, s=s: kq_chunk("q", m, s + 1))
    # partial out-proj (pairs 0-1) of the last s-block during its last item
    for et in range(0, NE, 2):
        put(
            11, 4 + 2 * et,
            lambda et=et: out01_chunk(et),
            lambda et=et: out01_chunk(et + 1),
        )
    # out-proj(s-1) pairs during item 3s: normalize(3s-1) lands at unit 1 and
    # its DVE burst runs ~units 1-5, so start at unit 6 to keep the ps3
    # evacuations (also DVE) from stalling the rotation
    for s in range(1, NS):
        for et in range(0, NE, 2):
            put(
                3 * s, 6 + 2 * et,
                lambda s=s, et=et: out_chunk(s - 1, et),
                lambda s=s, et=et: out_chunk(s - 1, et + 1),
            )

    # --- minimal prefix: k chunk (m=0, block 0) + q chunk (m=0, s0) ---
    emit_group(lambda: kq_chunk("k", 0, 0), lambda: kq_chunk("q", 0, 0))

    # --- fused attention stream with 2-group exp lag across boundaries ---
    from collections import deque

    pend = deque()

    def drain_one(last=False):
        pi, pg, pex = pend.popleft()
        emit_ctx(pi, pg, pex)
        if pg == NT - 1:
            normalize(pi, last=last)

    for i in range(len(ITEMS)):
        for g in range(NT):
            ex = emit_sc_exp(i, g)
            specs = extras.get((i, g))
            if specs:
                emit_group(*specs)
            if len(pend) == 3:
                drain_one()
            pend.append((i, g, ex))
    while pend:
        drain_one(last=(len(pend) == 1))
    for et in range(0, NE, 2):
        emit_group(
            lambda et=et: out2_chunk(et),
            lambda et=et: out2_chunk(et + 1, alt_pool=True),
        )


def build_nc():
    nc = bacc.Bacc("TRN2", target_bir_lowering=False, debug=False)
    hs = nc.dram_tensor("hs", [E, S], BF16, kind="ExternalInput")
    wqkv = nc.dram_tensor("wqkv", [E, 3 * HD], BF16, kind="ExternalInput")
    biases = nc.dram_tensor("biases", [128, 397], F32, kind="ExternalInput")
    woT = nc.dram_tensor("woT", [HD, E], BF16, kind="ExternalInput")
    outT = nc.dram_tensor("outT", [E, S], F32, kind="ExternalOutput")

    with tile.TileContext(nc) as tc:
        mha_tile(
            tc,
            hs[:, :], wqkv[:, :], biases[:, :], woT[:, :], outT[:, :],
        )
    nc.compile()
    return nc


def make_core_inputs(inputs: dict) -> list[dict]:
    """Full inputs -> per-core input maps (core c: batch c//2, head-group c%2)."""
    hsf = np.ascontiguousarray(np.asarray(inputs["hidden_state"], dtype=np.float32))
    Wq = np.asarray(inputs["Wq"], dtype=np.float32)
    Wk = np.asarray(inputs["Wk"], dtype=np.float32)
    Wv = np.asarray(inputs["Wv"], dtype=np.float32)
    Wo = np.asarray(inputs["Wo"], dtype=np.float32)
    bq = np.asarray(inputs["bq"], dtype=np.float32)
    bk = np.asarray(inputs["bk"], dtype=np.float32)
    bv = np.asarray(inputs["bv"], dtype=np.float32)
    bo = np.asarray(inputs["bo"], dtype=np.float32)

    maps = []
    for c in range(8):
        b, g = c // 2, c % 2
        hsl = slice(NH * g, NH * (g + 1))
        fsl = slice(HD * g, HD * (g + 1))
        wq_c = Wq[hsl].transpose(1, 0, 2).reshape(E, HD)
        wk_c = Wk[hsl].transpose(1, 0, 2).reshape(E, HD)
        wv_c = Wv[hsl].transpose(1, 0, 2).reshape(E, HD)
        # packed bias tile [128, 397]: bq(3) | bk(3) | bo/2(6) | bv-bcast | 0
        bias_t = np.zeros((128, 397), dtype=np.float32)
        bias_t[:, 0:3] = bq[hsl].reshape(HD).reshape(3, 128).T
        bias_t[:, 3:6] = bk[hsl].reshape(HD).reshape(3, 128).T
        bias_t[:, 6:12] = (bo / 2.0).reshape(6, 128).T
        bias_t[:, 12 : 12 + HD] = bv[hsl].reshape(HD)[None, :]
        bf16 = ml_dtypes.bfloat16
        maps.append(
            {
                "hs": np.ascontiguousarray(hsf[b].astype(bf16)),
                "wqkv": np.ascontiguousarray(
                    np.concatenate([wq_c, wk_c, wv_c], axis=1).astype(bf16)
                ),
                "biases": bias_t,
                "woT": np.ascontiguousarray(Wo[:, fsl].T.astype(bf16)),
            }
        )
    return maps


def combine_outputs(core_outs: list) -> np.ndarray:
    """Per-core outT partials -> full [B, E, S] output."""
    return np.stack(
        [core_outs[2 * b]["outT"] + core_outs[2 * b + 1]["outT"] for b in range(B)]
    ).astype(np.float32)


from concourse.bass_utils import run_bass_kernel_spmd

N_CORES = 8
_NC_CACHE = None


def _get_nc():
    global _NC_CACHE
    if _NC_CACHE is None:
        _NC_CACHE = build_nc()
    return _NC_CACHE


def kernel(**inputs) -> np.ndarray:
    """Full-input entry point: shard across 8 cores, run, unshard."""
    maps = make_core_inputs(inputs)
    nc = _get_nc()
    res = run_bass_kernel_spmd(nc, maps, core_ids=list(range(N_CORES)))
    outs = res.results
    return np.stack(
        [outs[2 * b]["outT"] + outs[2 * b + 1]["outT"] for b in range(B)]
    ).astype(np.float32)
